# revision 20
# baseline (speedup 1.0000x reference)
"""Multi-head attention Trainium2 kernel (overlap-optimized).

Problem: B=4, S=2048, D_MODEL=1024, H=16 heads, d_k=d_v=64.

Sharding (8 cores, no collectives): core c handles batch b=c//2 and head
group g=c%2 (8 heads). Each core computes its 8 heads' attention and the
partial output projection ctx @ Wo[g's rows]; the host sums the two
head-group partials per batch and adds the (folded) biases.

Math notes:
 - bk drops out of softmax exactly; bv and bo fold into a host-side row
   vector bo_eff = bo + bv@Wo; softmax without max subtraction (scores are
   small); matmuls in bf16 with fp32 PSUM accumulation.

Engine budget per core (theory): PE 273us (scores 55 + ctx 109 + projections
109), ACT 264us (exp of 33.5M score elements, its only job), DVE ~95us,
GPSIMD ~26us.  The emission order software-pipelines projections under the
exp stream so PE and ACT overlap nearly fully:

 - x inputs arrive bf16 from the host (no device-side convert).
 - K-proj runs in two passes of 2 head-chunks each (x streamed per pass,
   re-fetched), so the first score matmuls unlock after ~18us.
 - Attention sweeps q-chunks (waves): per wave, scores+exp for the 4 head
   pairs with ctx trailing one pair behind bounds live exp tiles to ~40.
 - ALL matmuls are split into 64-row contraction halves on alternating PE
   row groups: LDWEIGHTS pulls ahead into the idle row group, and the two
   half-chains run concurrently (without this, per-matmul weight loads
   serialize and PE runs ~1.5x slower).  Halves merge on DVE (copy + add).
 - ctx lhsT = [vh | ones] (65 cols); row 64 of the merged PSUM is the
   softmax denominator Z.  1/Z via DVE reciprocal_approx_fast (SBUF source
   only - it returns garbage from PSUM), GPSIMD broadcast + multiply.
 - out-projection per q-tile group emitted as soon as the last pair's ctx
   for that q range is done.
"""

import numpy as np
import ml_dtypes

import concourse.bass as bass
import concourse.bacc as bacc
import concourse.mybir as mybir
import concourse.tile as tile
from concourse.bass import ts

BF16 = mybir.dt.bfloat16
F32 = mybir.dt.float32

import os
RECIP_MODE = os.environ.get("RECIP_MODE", "approx")  # approx | exact
GPSIMD_MUL = os.environ.get("GPSIMD_MUL", "0") == "1"

D_MODEL, D_K, D_V, N_HEADS = 1024, 64, 64, 16
B, S = 4, 2048
N_CORES = 8
NH = 8            # heads per core
HD = NH * D_V     # 512
T = S
DC = 8            # D_MODEL / 128
TCN = 4           # t chunks of 512
SCN = 16          # s tiles of 128
QCN = 4           # q chunks of 512
HCN = 4           # hd chunks of 128 (one head pair each)
EXP_BUFS = 31


def build_nc(reps: int = 1, phases: str = "all"):
    nc = bacc.Bacc("TRN2", target_bir_lowering=False, debug=False)

    xq_d = nc.dram_tensor("xq_t", [128, DC, T], BF16, kind="ExternalInput")
    xk_d = nc.dram_tensor("xk_t", [128, DC, T], BF16, kind="ExternalInput")
    xv_d = nc.dram_tensor("xv_t", [128, DC, T], BF16, kind="ExternalInput")
    wq_d = nc.dram_tensor("wq", [128, DC, HD], BF16, kind="ExternalInput")
    wk_d = nc.dram_tensor("wk", [128, DC, HD], BF16, kind="ExternalInput")
    wv_d = nc.dram_tensor("wv", [128, DC, HD], BF16, kind="ExternalInput")
    wo_d = nc.dram_tensor("wo", [128, HCN, D_MODEL], BF16, kind="ExternalInput")
    bq_d = nc.dram_tensor("bq", [128, HCN], F32, kind="ExternalInput")
    out_d = nc.dram_tensor("out", [SCN, 128, D_MODEL], F32, kind="ExternalOutput")

    with tile.TileContext(nc) as tc:
        def body():
            emit_body(nc, tc, xq_d, xk_d, xv_d, wq_d, wk_d, wv_d, wo_d, bq_d, out_d, phases)

        if reps == 1:
            body()
        else:
            with tc.For_i(0, reps, 1):
                body()
    nc.compile()
    return nc


def emit_body(nc, tc, xq_d, xk_d, xv_d, wq_d, wk_d, wv_d, wo_d, bq_d, out_d, phases="all"):
    import contextlib

    ctx = contextlib.ExitStack()
    with ctx:
        wpool = ctx.enter_context(tc.tile_pool(name="wpool", bufs=1))
        qkpool = ctx.enter_context(tc.tile_pool(name="qkpool", bufs=1))
        vpool = ctx.enter_context(tc.tile_pool(name="vpool", bufs=1))
        cpool = ctx.enter_context(tc.tile_pool(name="cpool", bufs=1))
        xst = ctx.enter_context(tc.tile_pool(name="xst", bufs=2))
        vst = ctx.enter_context(tc.tile_pool(name="vst", bufs=2))
        expool = ctx.enter_context(tc.tile_pool(name="expool", bufs=EXP_BUFS))
        zpool = ctx.enter_context(tc.tile_pool(name="zpool", bufs=2))
        opool = ctx.enter_context(tc.tile_pool(name="opool", bufs=2))
        ps = ctx.enter_context(tc.tile_pool(name="ps", bufs=2, space="PSUM"))
        cp = ctx.enter_context(tc.tile_pool(name="cp", bufs=4, space="PSUM"))
        pq = cp

        wq_sb = wpool.tile([128, DC, HD], BF16, tag="wq")
        wk_sb = wpool.tile([128, DC, HD], BF16, tag="wk")
        wv_sb = wpool.tile([128, DC, HD], BF16, tag="wv")
        wo_sb = wpool.tile([128, HCN, D_MODEL], BF16, tag="wo")
        bq_sb = wpool.tile([128, HCN], F32, tag="bq")

        nc.sync.dma_start(wk_sb[:], wk_d[:])
        nc.sync.dma_start(wv_sb[:], wv_d[:])
        nc.sync.dma_start(wq_sb[:], wq_d[:])
        nc.sync.dma_start(wo_sb[:], wo_d[:])
        nc.sync.dma_start(bq_sb[:], bq_d[:])

        qhT = qkpool.tile([128, HCN, T], BF16, tag="qhT")  # [hd%128, hc, t]
        khT = qkpool.tile([128, HCN, T], BF16, tag="khT")
        vha = vpool.tile([128, SCN, NH, D_V + 1], BF16, tag="vha")  # [s%128, s//128, h, dv|1]
        ctxT = cpool.tile([128, HCN, T], BF16, tag="ctxT")

        nc.vector.memset(vha[:, :, :, D_V : D_V + 1], 1.0)

        exp_tiles = {}

        def _proj_mms(w_sb, hc, xt):
            # split-K halves on alternating row groups: LDWEIGHTS of one half
            # pulls ahead under the other half's matmul, and the two 64-row
            # accumulation chains run concurrently on the row-tiled PE.
            pa = pq.tile([128, 512], F32, tag="u", name="pa_t")
            pb = pq.tile([128, 512], F32, tag="u", name="pb_t")
            for dc in range(DC):
                nc.tensor.matmul(
                    pa[:],
                    lhsT=w_sb[0:64, dc, ts(hc, 128)],
                    rhs=xt[0:64, dc, :],
                    start=(dc == 0),
                    stop=(dc == DC - 1),
                )
                nc.tensor.matmul(
                    pb[:],
                    lhsT=w_sb[64:128, dc, ts(hc, 128)],
                    rhs=xt[64:128, dc, :],
                    start=(dc == 0),
                    stop=(dc == DC - 1),
                )
            return pa, pb

        def kproj(hcg, tc_i):
            xt = xst.tile([128, DC, 512], BF16, tag="x", name="xk_t")
            nc.sync.dma_start(xt[:], xk_d[:, :, ts(tc_i, 512)])
            for hc in (2 * hcg, 2 * hcg + 1):
                pa, pb = _proj_mms(wk_sb, hc, xt)
                t = zpool.tile([128, 512], F32, tag="mg", name="kmg_t")
                nc.vector.tensor_copy(t[:], pb[:])
                nc.vector.tensor_add(khT[:, hc, ts(tc_i, 512)], pa[:], t[:])

        def qproj(hcg, tc_i):
            xt = xst.tile([128, DC, 512], BF16, tag="x", name="xq_t")
            nc.sync.dma_start(xt[:], xq_d[:, :, ts(tc_i, 512)])
            for hc in (2 * hcg, 2 * hcg + 1):
                pa, pb = _proj_mms(wq_sb, hc, xt)
                t = zpool.tile([128, 512], F32, tag="mg", name="qmg_t")
                nc.vector.tensor_scalar_add(t[:], pb[:], bq_sb[:, hc : hc + 1])
                nc.vector.tensor_add(qhT[:, hc, ts(tc_i, 512)], pa[:], t[:])

        def vproj(sc):
            xt = vst.tile([128, DC, 128], BF16, tag="xv", name="xv_t")
            nc.sync.dma_start(xt[:], xv_d[:, :, ts(sc, 128)])
            pa = pq.tile([128, 512], F32, tag="u", name="pva_t")
            pb = pq.tile([128, 512], F32, tag="u", name="pvb_t")
            for dc in range(DC):
                nc.tensor.matmul(
                    pa[:],
                    lhsT=xt[0:64, dc, :],
                    rhs=wv_sb[0:64, dc, :],
                    start=(dc == 0),
                    stop=(dc == DC - 1),
                )
                nc.tensor.matmul(
                    pb[:],
                    lhsT=xt[64:128, dc, :],
                    rhs=wv_sb[64:128, dc, :],
                    start=(dc == 0),
                    stop=(dc == DC - 1),
                )
            t = zpool.tile([128, 512], F32, tag="mg", name="vmg_t")
            nc.vector.tensor_copy(t[:], pb[:])
            nc.vector.tensor_add(
                vha[:, sc, :, 0:D_V],
                pa[:].rearrange("p (h d) -> p h d", d=D_V),
                t[:].rearrange("p (h d) -> p h d", d=D_V),
            )

        def scores_exp(p, qc):
            for sc in range(SCN):
                s_ps = ps.tile([128, 1024], F32, tag="ps", name="s_ps")
                for hl in range(2):
                    pb = hl * 64
                    nc.tensor.matmul(
                        s_ps[:, ts(hl, 512)],
                        lhsT=khT[pb : pb + 64, p, ts(sc, 128)],
                        rhs=qhT[pb : pb + 64, p, ts(qc, 512)],
                        start=True,
                        stop=True,
                    )
                if "peonly" in phases:
                    # probe: no ACT at all; tiny DVE sliver keeps mms alive
                    sv = zpool.tile([1, 8], F32, tag="sv", name="sv_t")
                    nc.vector.tensor_copy(sv[:], s_ps[0:1, 0:8])
                    exp_tiles[(p, sc, qc)] = None
                    continue
                e = expool.tile([128, 1024], BF16, tag="exp", name="exp_t")
                if "lowact" in phases and sc > 0:
                    # probe: tiny exp keeps deps alive but ~zero ACT time
                    nc.scalar.activation(
                        e[:, 0:8], s_ps[:, 0:8],
                        mybir.ActivationFunctionType.Exp, scale=0.125,
                    )
                else:
                    nc.scalar.activation(
                        e[:], s_ps[:], mybir.ActivationFunctionType.Exp, scale=0.125
                    )
                exp_tiles[(p, sc, qc)] = e

        def ctx_one(p, qc):
            for hl in range(2):
                h = 2 * p + hl
                pb = hl * 64
                c_a = cp.tile([128, 512], F32, tag="u", name="ca_t")
                c_b = cp.tile([128, 512], F32, tag="u", name="cb_t")
                for sc in range(SCN):
                    e = exp_tiles[(p, sc, qc)]
                    rhs = (
                        qhT[:, p, ts(qc, 512)] if "peonly" in phases
                        else e[:, ts(hl, 512)]
                    )
                    nc.tensor.matmul(
                        c_a[0 : D_V + 1, :],
                        lhsT=vha[0:64, sc, h, :],
                        rhs=rhs[0:64, :],
                        start=(sc == 0),
                        stop=(sc == SCN - 1),
                    )
                    nc.tensor.matmul(
                        c_b[0 : D_V + 1, :],
                        lhsT=vha[64:128, sc, h, :],
                        rhs=rhs[64:128, :],
                        start=(sc == 0),
                        stop=(sc == SCN - 1),
                    )
                cb_sb = zpool.tile([D_V + 1, 512], F32, tag="cbs", name="cbs_t")
                nc.vector.tensor_copy(cb_sb[:], c_b[0 : D_V + 1, :])
                t1 = zpool.tile([D_V, 512], F32, tag="t1", name="t1_t")
                nc.vector.tensor_add(t1[:], c_a[0:D_V, :], cb_sb[0:D_V, :])
                # Z into its own tile at partition 0 (reciprocal_approx_fast
                # returns garbage for nonzero base partitions / PSUM sources)
                tz = zpool.tile([1, 512], F32, tag="tz", name="tz_t")
                nc.vector.tensor_add(tz[:], c_a[D_V : D_V + 1, :], cb_sb[D_V : D_V + 1, :])
                if "norecip" in phases:
                    # probe: skip the Z-normalize chain (wrong values)
                    nc.vector.tensor_copy(
                        ctxT[pb : pb + 64, p, ts(qc, 512)], t1[0:D_V, :]
                    )
                else:
                    rz = zpool.tile([1, 512], F32, tag="rz", name="rz_t")
                    if RECIP_MODE == "exact":
                        nc.vector.reciprocal(rz[:], tz[:])
                    else:
                        nc.vector.reciprocal_approx_fast(rz[:], tz[:])
                    bc = zpool.tile([64, 512], F32, tag="bc", name="bc_t")
                    nc.gpsimd.partition_broadcast(bc[:], rz[:], channels=64)
                    if GPSIMD_MUL:
                        nc.gpsimd.tensor_mul(
                            ctxT[pb : pb + 64, p, ts(qc, 512)], t1[0:D_V, :], bc[:]
                        )
                    else:
                        nc.vector.tensor_mul(
                            ctxT[pb : pb + 64, p, ts(qc, 512)], t1[0:D_V, :], bc[:]
                        )
            for sc in range(SCN):
                del exp_tiles[(p, sc, qc)]

        def outproj(qt):
            for d2 in range(2):
                pa = pq.tile([128, 512], F32, tag="u", name="poa_t")
                pb2 = pq.tile([128, 512], F32, tag="u", name="pob_t")
                for hc in range(HCN):
                    nc.tensor.matmul(
                        pa[:],
                        lhsT=ctxT[0:64, hc, ts(qt, 128)],
                        rhs=wo_sb[0:64, hc, ts(d2, 512)],
                        start=(hc == 0),
                        stop=(hc == HCN - 1),
                    )
                    nc.tensor.matmul(
                        pb2[:],
                        lhsT=ctxT[64:128, hc, ts(qt, 128)],
                        rhs=wo_sb[64:128, hc, ts(d2, 512)],
                        start=(hc == 0),
                        stop=(hc == HCN - 1),
                    )
                t = zpool.tile([128, 512], F32, tag="mg", name="omg_t")
                nc.vector.tensor_copy(t[:], pb2[:])
                o_sb = opool.tile([128, 512], F32, tag="o", name="o_sb")
                nc.vector.tensor_add(o_sb[:], pa[:], t[:])
                nc.sync.dma_start(out_d[qt, :, ts(d2, 512)], o_sb[:])

        # ------------------- emission schedule -------------------
        # Prologue: K pass A (hc 0-1), first Q chunk, V interleaved.
        kproj(0, 0)
        vproj(0)
        kproj(0, 1)
        vproj(1)
        kproj(0, 2)
        vproj(2)
        kproj(0, 3)
        vproj(3)
        qproj(0, 0)
        vproj(4)
        # First scores for pairs 0-1 while K pass B runs.
        scores_exp(0, 0)
        kproj(1, 0)
        vproj(5)
        kproj(1, 1)
        vproj(6)
        scores_exp(1, 0)
        kproj(1, 2)
        vproj(7)
        kproj(1, 3)
        vproj(8)
        qproj(1, 0)
        for sc in range(9, SCN):
            vproj(sc)
        scores_exp(2, 0)
        ctx_one(0, 0)
        scores_exp(3, 0)
        ctx_one(1, 0)

        # Steady-state waves over remaining q chunks.
        for qc in range(1, QCN):
            qproj(0, qc)
            scores_exp(0, qc)
            ctx_one(2, qc - 1)
            scores_exp(1, qc)
            ctx_one(3, qc - 1)
            if qc >= 2:
                for qt in range(4 * (qc - 2), 4 * (qc - 1)):
                    outproj(qt)
            qproj(1, qc)
            scores_exp(2, qc)
            ctx_one(0, qc)
            scores_exp(3, qc)
            ctx_one(1, qc)

        ctx_one(2, QCN - 1)
        ctx_one(3, QCN - 1)
        for qt in range(8, SCN):
            outproj(qt)


# ---------------------------------------------------------------------------
# host side
# ---------------------------------------------------------------------------

_NC_CACHE = {}


def _get_nc(reps: int = 1):
    if reps not in _NC_CACHE:
        _NC_CACHE[reps] = build_nc(reps)
    return _NC_CACHE[reps]


def _to_bf16(a):
    return np.ascontiguousarray(a).astype(ml_dtypes.bfloat16)


def make_in_maps(q, k, v, Wq, bq, Wk, bk, Wv, bv, Wo, bo):
    """Build the per-core input maps (host-side sharding + layout)."""
    in_maps = []
    for c in range(N_CORES):
        b = c // 2
        hg = c % 2
        hs = slice(hg * NH, hg * NH + NH)

        def xt(x):
            # (S, D) -> [p, dc, t] bf16 with D = dc*128 + p
            return _to_bf16(
                np.asarray(x, np.float32).T.reshape(DC, 128, T).transpose(1, 0, 2)
            )

        def wproj(W):
            # (8, 1024, 64) -> [p, dc, hd]  (hd = h*64+dv, D = dc*128+p)
            Wc = np.asarray(W[hs], np.float32).transpose(1, 0, 2).reshape(D_MODEL, HD)
            return _to_bf16(Wc.reshape(DC, 128, HD).transpose(1, 0, 2))

        wo_c = np.asarray(Wo[hg * HD : (hg + 1) * HD], np.float32)  # (512, 1024)
        bq_c = np.asarray(bq[hs], np.float32).reshape(HD)  # (512,)

        in_maps.append(
            {
                "xq_t": xt(q[b]),
                "xk_t": xt(k[b]),
                "xv_t": xt(v[b]),
                "wq": wproj(Wq),
                "wk": wproj(Wk),
                "wv": wproj(Wv),
                "wo": _to_bf16(wo_c.reshape(HCN, 128, D_MODEL).transpose(1, 0, 2)),
                "bq": np.ascontiguousarray(bq_c.reshape(HCN, 128).T),
            }
        )
    return in_maps


def combine_outputs(results, bv, Wo, bo):
    """results: list of 8 dicts with 'out' (16,128,1024). Returns (B,S,D)."""
    bo_eff = np.asarray(bo, np.float32) + np.asarray(bv, np.float32).reshape(-1) @ np.asarray(
        Wo, np.float32
    )
    out = np.empty((B, S, D_MODEL), np.float32)
    for b in range(B):
        p0 = results[2 * b]["out"].reshape(S, D_MODEL)
        p1 = results[2 * b + 1]["out"].reshape(S, D_MODEL)
        out[b] = p0 + p1 + bo_eff
    return out


def kernel(q, k, v, Wq, bq, Wk, bk, Wv, bv, Wo, bo):
    from concourse.bass_utils import run_bass_kernel_spmd

    nc = _get_nc(1)
    in_maps = make_in_maps(q, k, v, Wq, bq, Wk, bk, Wv, bv, Wo, bo)
    res = run_bass_kernel_spmd(nc, in_maps, core_ids=list(range(N_CORES)))
    return combine_outputs(res.results, bv, Wo, bo)


# revision 21
# speedup vs baseline: 1.6189x; 1.6189x over previous
"""Multi-head attention Trainium2 kernel (overlap-optimized).

Problem: B=4, S=2048, D_MODEL=1024, H=16 heads, d_k=d_v=64.

Sharding (8 cores, no collectives): core c handles batch b=c//2 and head
group g=c%2 (8 heads). Each core computes its 8 heads' attention and the
partial output projection ctx @ Wo[g's rows]; the host sums the two
head-group partials per batch and adds the (folded) biases.

Math notes:
 - bk drops out of softmax exactly; bv and bo fold into a host-side row
   vector bo_eff = bo + bv@Wo; softmax without max subtraction (scores are
   small); matmuls in bf16 with fp32 PSUM accumulation.

Structure (v3):
 - x inputs arrive bf16 from the host (no device-side convert, half the DMA).
 - K-proj runs in two passes of 2 head-chunks each (x streamed, re-fetched),
   so the first score matmuls unlock after ~18us; Q-proj per-512-token chunk.
 - Attention sweeps q-chunks (waves): scores+exp for the 4 head pairs with
   ctx trailing, bounding live exp tiles to ~32; out-projection per q-tile
   group as soon as the last pair's ctx for that range is done.  ScalarE does
   ONLY the 256 exp instructions (~260us busy); merges/copies live on DVE.
 - ctx: K=128 matmuls, lhsT = vha = [vh | ones] (65 cols); PSUM row 64 is
   the softmax denominator Z.  Z is bounced to an SBUF tile at partition 0
   because reciprocal_approx_fast returns garbage for PSUM sources and
   nonzero base partitions; then GPSIMD broadcast + DVE multiply.
 - Measured per-matmul slot cost is ~290ns at N=512 under sustained load
   (~2.0GHz effective); the kernel sits at that wall: 1536 slots ~ 400us.
   Splitting contractions into 64-row halves (tile_position pairs) did NOT
   yield row-group concurrency on this hardware path and the extra DVE
   merges made it slower -- keep single K=128 matmuls.
"""

import os

import numpy as np
import ml_dtypes

import concourse.bass as bass
import concourse.bacc as bacc
import concourse.mybir as mybir
import concourse.tile as tile
from concourse.bass import ts

BF16 = mybir.dt.bfloat16
F32 = mybir.dt.float32

RECIP_MODE = os.environ.get("RECIP_MODE", "approx")  # approx | exact

D_MODEL, D_K, D_V, N_HEADS = 1024, 64, 64, 16
B, S = 4, 2048
N_CORES = 8
NH = 8            # heads per core
HD = NH * D_V     # 512
T = S
DC = 8            # D_MODEL / 128
TCN = 4           # t chunks of 512
SCN = 16          # s tiles of 128
QCN = 4           # q chunks of 512
HCN = 4           # hd chunks of 128 (one head pair each)
EXP_BUFS = 34


def build_nc(reps: int = 1, phases: str = "all"):
    nc = bacc.Bacc("TRN2", target_bir_lowering=False, debug=False)

    xq_d = nc.dram_tensor("xq_t", [128, DC, T], BF16, kind="ExternalInput")
    xk_d = nc.dram_tensor("xk_t", [128, DC, T], BF16, kind="ExternalInput")
    xv_d = nc.dram_tensor("xv_t", [128, DC, T], BF16, kind="ExternalInput")
    wq_d = nc.dram_tensor("wq", [128, DC, HD], BF16, kind="ExternalInput")
    wk_d = nc.dram_tensor("wk", [128, DC, HD], BF16, kind="ExternalInput")
    wv_d = nc.dram_tensor("wv", [128, DC, HD], BF16, kind="ExternalInput")
    wo_d = nc.dram_tensor("wo", [128, HCN, D_MODEL], BF16, kind="ExternalInput")
    bq_d = nc.dram_tensor("bq", [128, HCN], F32, kind="ExternalInput")
    out_d = nc.dram_tensor("out", [SCN, 128, D_MODEL], F32, kind="ExternalOutput")

    with tile.TileContext(nc) as tc:
        def body():
            emit_body(nc, tc, xq_d, xk_d, xv_d, wq_d, wk_d, wv_d, wo_d, bq_d, out_d, phases)

        if reps == 1:
            body()
        else:
            with tc.For_i(0, reps, 1):
                body()
    nc.compile()
    return nc


def emit_body(nc, tc, xq_d, xk_d, xv_d, wq_d, wk_d, wv_d, wo_d, bq_d, out_d, phases="all"):
    import contextlib

    ctx = contextlib.ExitStack()
    with ctx:
        wpool = ctx.enter_context(tc.tile_pool(name="wpool", bufs=1))
        qkpool = ctx.enter_context(tc.tile_pool(name="qkpool", bufs=1))
        vpool = ctx.enter_context(tc.tile_pool(name="vpool", bufs=1))
        cpool = ctx.enter_context(tc.tile_pool(name="cpool", bufs=1))
        xst = ctx.enter_context(tc.tile_pool(name="xst", bufs=2))
        vst = ctx.enter_context(tc.tile_pool(name="vst", bufs=2))
        expool = ctx.enter_context(tc.tile_pool(name="expool", bufs=EXP_BUFS))
        zpool = ctx.enter_context(tc.tile_pool(name="zpool", bufs=2))
        opool = ctx.enter_context(tc.tile_pool(name="opool", bufs=2))
        ps = ctx.enter_context(tc.tile_pool(name="ps", bufs=2, space="PSUM"))
        cp = ctx.enter_context(tc.tile_pool(name="cp", bufs=4, space="PSUM"))

        wq_sb = wpool.tile([128, DC, HD], BF16, tag="wq")
        wk_sb = wpool.tile([128, DC, HD], BF16, tag="wk")
        wv_sb = wpool.tile([128, DC, HD], BF16, tag="wv")
        wo_sb = wpool.tile([128, HCN, D_MODEL], BF16, tag="wo")
        bq_sb = wpool.tile([128, HCN], F32, tag="bq")

        nc.sync.dma_start(wk_sb[:], wk_d[:])
        nc.sync.dma_start(wv_sb[:], wv_d[:])
        nc.sync.dma_start(wq_sb[:], wq_d[:])
        nc.sync.dma_start(wo_sb[:], wo_d[:])
        nc.sync.dma_start(bq_sb[:], bq_d[:])

        qhT = qkpool.tile([128, HCN, T], BF16, tag="qhT")  # [hd%128, hc, t]
        khT = qkpool.tile([128, HCN, T], BF16, tag="khT")
        vha = vpool.tile([128, SCN, NH, D_V + 1], BF16, tag="vha")  # [s%128, s//128, h, dv|1]
        ctxT = cpool.tile([128, HCN, T], BF16, tag="ctxT")

        nc.vector.memset(vha[:, :, :, D_V : D_V + 1], 1.0)

        exp_tiles = {}

        def kproj(hcg, tc_i):
            xt = xst.tile([128, DC, 512], BF16, tag="x", name="xk_t")
            nc.sync.dma_start(xt[:], xk_d[:, :, ts(tc_i, 512)])
            for hc in (2 * hcg, 2 * hcg + 1):
                pt = cp.tile([128, 512], F32, tag="u", name="kp_t")
                for dc in range(DC):
                    nc.tensor.matmul(
                        pt[:],
                        lhsT=wk_sb[:, dc, ts(hc, 128)],
                        rhs=xt[:, dc, :],
                        start=(dc == 0),
                        stop=(dc == DC - 1),
                    )
                nc.vector.tensor_copy(khT[:, hc, ts(tc_i, 512)], pt[:])

        def qproj(hcg, tc_i):
            xt = xst.tile([128, DC, 512], BF16, tag="x", name="xq_t")
            nc.sync.dma_start(xt[:], xq_d[:, :, ts(tc_i, 512)])
            for hc in (2 * hcg, 2 * hcg + 1):
                pt = cp.tile([128, 512], F32, tag="u", name="qp_t")
                for dc in range(DC):
                    nc.tensor.matmul(
                        pt[:],
                        lhsT=wq_sb[:, dc, ts(hc, 128)],
                        rhs=xt[:, dc, :],
                        start=(dc == 0),
                        stop=(dc == DC - 1),
                    )
                nc.vector.tensor_scalar_add(
                    qhT[:, hc, ts(tc_i, 512)], pt[:], bq_sb[:, hc : hc + 1]
                )

        def vproj(sc):
            xt = vst.tile([128, DC, 128], BF16, tag="xv", name="xv_t")
            nc.sync.dma_start(xt[:], xv_d[:, :, ts(sc, 128)])
            pv = cp.tile([128, 512], F32, tag="u", name="pv_t")
            for dc in range(DC):
                nc.tensor.matmul(
                    pv[:],
                    lhsT=xt[:, dc, :],
                    rhs=wv_sb[:, dc, :],
                    start=(dc == 0),
                    stop=(dc == DC - 1),
                )
            nc.vector.tensor_copy(
                vha[:, sc, :, 0:D_V], pv[:].rearrange("p (h d) -> p h d", d=D_V)
            )

        def scores_exp(p, qc):
            for sc in range(SCN):
                s_ps = ps.tile([128, 1024], F32, tag="ps", name="s_ps")
                for hl in range(2):
                    pb = hl * 64
                    nc.tensor.matmul(
                        s_ps[:, ts(hl, 512)],
                        lhsT=khT[pb : pb + 64, p, ts(sc, 128)],
                        rhs=qhT[pb : pb + 64, p, ts(qc, 512)],
                        start=True,
                        stop=True,
                    )
                if "peonly" in phases:
                    sv = zpool.tile([1, 8], F32, tag="sv", name="sv_t")
                    nc.vector.tensor_copy(sv[:], s_ps[0:1, 0:8])
                    exp_tiles[(p, sc, qc)] = None
                    continue
                e = expool.tile([128, 1024], BF16, tag="exp", name="exp_t")
                if "lowact" in phases and sc > 0:
                    nc.scalar.activation(
                        e[:, 0:8], s_ps[:, 0:8],
                        mybir.ActivationFunctionType.Exp, scale=0.125,
                    )
                else:
                    nc.scalar.activation(
                        e[:], s_ps[:], mybir.ActivationFunctionType.Exp, scale=0.125
                    )
                exp_tiles[(p, sc, qc)] = e

        def ctx_one(p, qc):
            for hl in range(2):
                h = 2 * p + hl
                pb = hl * 64
                ct = cp.tile([128, 512], F32, tag="u", name="ct_t")
                for sc in range(SCN):
                    e = exp_tiles[(p, sc, qc)]
                    rhs = (
                        qhT[:, p, ts(qc, 512)] if "peonly" in phases
                        else e[:, ts(hl, 512)]
                    )
                    nc.tensor.matmul(
                        ct[0 : D_V + 1, :],
                        lhsT=vha[:, sc, h, :],
                        rhs=rhs,
                        start=(sc == 0),
                        stop=(sc == SCN - 1),
                    )
                if "norecip" in phases:
                    nc.vector.tensor_copy(
                        ctxT[pb : pb + 64, p, ts(qc, 512)], ct[0:D_V, :]
                    )
                else:
                    # Z to partition 0 in SBUF (approx recip needs both)
                    tz = zpool.tile([1, 512], F32, tag="tz", name="tz_t")
                    nc.vector.tensor_copy(tz[:], ct[D_V : D_V + 1, :])
                    rz = zpool.tile([1, 512], F32, tag="rz", name="rz_t")
                    if RECIP_MODE == "exact":
                        nc.vector.reciprocal(rz[:], tz[:])
                    else:
                        nc.vector.reciprocal_approx_fast(rz[:], tz[:])
                    bc = zpool.tile([64, 512], F32, tag="bc", name="bc_t")
                    nc.gpsimd.partition_broadcast(bc[:], rz[:], channels=64)
                    nc.vector.tensor_mul(
                        ctxT[pb : pb + 64, p, ts(qc, 512)], ct[0:D_V, :], bc[:]
                    )
            for sc in range(SCN):
                del exp_tiles[(p, sc, qc)]

        def outproj(qt):
            for d2 in range(2):
                po = cp.tile([128, 512], F32, tag="u", name="po_t")
                for hc in range(HCN):
                    nc.tensor.matmul(
                        po[:],
                        lhsT=ctxT[:, hc, ts(qt, 128)],
                        rhs=wo_sb[:, hc, ts(d2, 512)],
                        start=(hc == 0),
                        stop=(hc == HCN - 1),
                    )
                o_sb = opool.tile([128, 512], F32, tag="o", name="o_sb")
                nc.vector.tensor_copy(o_sb[:], po[:])
                nc.sync.dma_start(out_d[qt, :, ts(d2, 512)], o_sb[:])

        # ------------------- emission schedule -------------------
        kproj(0, 0)
        vproj(0)
        kproj(0, 1)
        vproj(1)
        kproj(0, 2)
        vproj(2)
        kproj(0, 3)
        vproj(3)
        qproj(0, 0)
        vproj(4)
        scores_exp(0, 0)
        kproj(1, 0)
        vproj(5)
        kproj(1, 1)
        vproj(6)
        scores_exp(1, 0)
        kproj(1, 2)
        vproj(7)
        kproj(1, 3)
        vproj(8)
        qproj(1, 0)
        for sc in range(9, SCN):
            vproj(sc)
        scores_exp(2, 0)
        ctx_one(0, 0)
        scores_exp(3, 0)
        ctx_one(1, 0)

        for qc in range(1, QCN):
            qproj(0, qc)
            scores_exp(0, qc)
            ctx_one(2, qc - 1)
            scores_exp(1, qc)
            ctx_one(3, qc - 1)
            if qc >= 2:
                for qt in range(4 * (qc - 2), 4 * (qc - 1)):
                    outproj(qt)
            qproj(1, qc)
            scores_exp(2, qc)
            ctx_one(0, qc)
            scores_exp(3, qc)
            ctx_one(1, qc)

        ctx_one(2, QCN - 1)
        ctx_one(3, QCN - 1)
        for qt in range(8, SCN):
            outproj(qt)


# ---------------------------------------------------------------------------
# host side
# ---------------------------------------------------------------------------

_NC_CACHE = {}


def _get_nc(reps: int = 1):
    if reps not in _NC_CACHE:
        _NC_CACHE[reps] = build_nc(reps)
    return _NC_CACHE[reps]


def _to_bf16(a):
    return np.ascontiguousarray(a).astype(ml_dtypes.bfloat16)


def make_in_maps(q, k, v, Wq, bq, Wk, bk, Wv, bv, Wo, bo):
    """Build the per-core input maps (host-side sharding + layout)."""
    in_maps = []
    for c in range(N_CORES):
        b = c // 2
        hg = c % 2
        hs = slice(hg * NH, hg * NH + NH)

        def xt(x):
            # (S, D) -> [p, dc, t] bf16 with D = dc*128 + p
            return _to_bf16(
                np.asarray(x, np.float32).T.reshape(DC, 128, T).transpose(1, 0, 2)
            )

        def wproj(W):
            # (8, 1024, 64) -> [p, dc, hd]  (hd = h*64+dv, D = dc*128+p)
            Wc = np.asarray(W[hs], np.float32).transpose(1, 0, 2).reshape(D_MODEL, HD)
            return _to_bf16(Wc.reshape(DC, 128, HD).transpose(1, 0, 2))

        wo_c = np.asarray(Wo[hg * HD : (hg + 1) * HD], np.float32)  # (512, 1024)
        bq_c = np.asarray(bq[hs], np.float32).reshape(HD)  # (512,)

        in_maps.append(
            {
                "xq_t": xt(q[b]),
                "xk_t": xt(k[b]),
                "xv_t": xt(v[b]),
                "wq": wproj(Wq),
                "wk": wproj(Wk),
                "wv": wproj(Wv),
                "wo": _to_bf16(wo_c.reshape(HCN, 128, D_MODEL).transpose(1, 0, 2)),
                "bq": np.ascontiguousarray(bq_c.reshape(HCN, 128).T),
            }
        )
    return in_maps


def combine_outputs(results, bv, Wo, bo):
    """results: list of 8 dicts with 'out' (16,128,1024). Returns (B,S,D)."""
    bo_eff = np.asarray(bo, np.float32) + np.asarray(bv, np.float32).reshape(-1) @ np.asarray(
        Wo, np.float32
    )
    out = np.empty((B, S, D_MODEL), np.float32)
    for b in range(B):
        p0 = results[2 * b]["out"].reshape(S, D_MODEL)
        p1 = results[2 * b + 1]["out"].reshape(S, D_MODEL)
        out[b] = p0 + p1 + bo_eff
    return out


def kernel(q, k, v, Wq, bq, Wk, bk, Wv, bv, Wo, bo):
    from concourse.bass_utils import run_bass_kernel_spmd

    nc = _get_nc(1)
    in_maps = make_in_maps(q, k, v, Wq, bq, Wk, bk, Wv, bv, Wo, bo)
    res = run_bass_kernel_spmd(nc, in_maps, core_ids=list(range(N_CORES)))
    return combine_outputs(res.results, bv, Wo, bo)


# revision 22
# speedup vs baseline: 1.6932x; 1.0459x over previous
"""Multi-head attention Trainium2 kernel (overlap-optimized).

Problem: B=4, S=2048, D_MODEL=1024, H=16 heads, d_k=d_v=64.

Sharding (8 cores, no collectives): core c handles batch b=c//2 and head
group g=c%2 (8 heads). Each core computes its 8 heads' attention and the
partial output projection ctx @ Wo[g's rows]; the host sums the two
head-group partials per batch and adds the (folded) biases.

Math notes:
 - bk drops out of softmax exactly; bv and bo fold into a host-side row
   vector bo_eff = bo + bv@Wo; softmax without max subtraction (scores are
   small); matmuls in bf16 with fp32 PSUM accumulation.

Structure (v3):
 - x inputs arrive bf16 from the host (no device-side convert, half the DMA).
 - K-proj runs in two passes of 2 head-chunks each (x streamed, re-fetched),
   so the first score matmuls unlock after ~18us; Q-proj per-512-token chunk.
 - Attention sweeps q-chunks (waves): scores+exp for the 4 head pairs with
   ctx trailing, bounding live exp tiles to ~32; out-projection per q-tile
   group as soon as the last pair's ctx for that range is done.  ScalarE does
   ONLY the 256 exp instructions (~260us busy); merges/copies live on DVE.
 - ctx: K=128 matmuls, lhsT = vha = [vh | ones] (65 cols); PSUM row 64 is
   the softmax denominator Z.  Z is bounced to an SBUF tile at partition 0
   because reciprocal_approx_fast returns garbage for PSUM sources and
   nonzero base partitions; then GPSIMD broadcast + DVE multiply.
 - Measured per-matmul slot cost is ~290ns at N=512 under sustained load
   (~2.0GHz effective); the kernel sits at that wall: 1536 slots ~ 400us.
   Splitting contractions into 64-row halves (tile_position pairs) did NOT
   yield row-group concurrency on this hardware path and the extra DVE
   merges made it slower -- keep single K=128 matmuls.
"""

import os

import numpy as np
import ml_dtypes

import concourse.bass as bass
import concourse.bacc as bacc
import concourse.mybir as mybir
import concourse.tile as tile
from concourse.bass import ts

BF16 = mybir.dt.bfloat16
F32 = mybir.dt.float32

RECIP_MODE = os.environ.get("RECIP_MODE", "approx")  # approx | exact

D_MODEL, D_K, D_V, N_HEADS = 1024, 64, 64, 16
B, S = 4, 2048
N_CORES = 8
NH = 8            # heads per core
HD = NH * D_V     # 512
T = S
DC = 8            # D_MODEL / 128
TCN = 4           # t chunks of 512
SCN = 16          # s tiles of 128
QCN = 4           # q chunks of 512
HCN = 4           # hd chunks of 128 (one head pair each)
EXP_BUFS = 34


def build_nc(reps: int = 1, phases: str = "all"):
    nc = bacc.Bacc("TRN2", target_bir_lowering=False, debug=False)

    xq_d = nc.dram_tensor("xq_t", [128, DC, T], BF16, kind="ExternalInput")
    xk_d = nc.dram_tensor("xk_t", [128, DC, T], BF16, kind="ExternalInput")
    xv_d = nc.dram_tensor("xv_t", [128, DC, T], BF16, kind="ExternalInput")
    wq_d = nc.dram_tensor("wq", [128, DC, HD], BF16, kind="ExternalInput")
    wk_d = nc.dram_tensor("wk", [128, DC, HD], BF16, kind="ExternalInput")
    wv_d = nc.dram_tensor("wv", [128, DC, HD], BF16, kind="ExternalInput")
    wo_d = nc.dram_tensor("wo", [128, HCN, D_MODEL], BF16, kind="ExternalInput")
    bq_d = nc.dram_tensor("bq", [128, HCN], F32, kind="ExternalInput")
    out_d = nc.dram_tensor("out", [SCN, 128, D_MODEL], F32, kind="ExternalOutput")

    with tile.TileContext(nc) as tc:
        def body():
            emit_body(nc, tc, xq_d, xk_d, xv_d, wq_d, wk_d, wv_d, wo_d, bq_d, out_d, phases)

        if reps == 1:
            body()
        else:
            with tc.For_i(0, reps, 1):
                body()
    nc.compile()
    return nc


def emit_body(nc, tc, xq_d, xk_d, xv_d, wq_d, wk_d, wv_d, wo_d, bq_d, out_d, phases="all"):
    import contextlib

    ctx = contextlib.ExitStack()
    with ctx:
        wpool = ctx.enter_context(tc.tile_pool(name="wpool", bufs=1))
        qkpool = ctx.enter_context(tc.tile_pool(name="qkpool", bufs=1))
        vpool = ctx.enter_context(tc.tile_pool(name="vpool", bufs=1))
        cpool = ctx.enter_context(tc.tile_pool(name="cpool", bufs=1))
        xst = ctx.enter_context(tc.tile_pool(name="xst", bufs=2))
        vst = ctx.enter_context(tc.tile_pool(name="vst", bufs=2))
        expool = ctx.enter_context(tc.tile_pool(name="expool", bufs=EXP_BUFS))
        zpool = ctx.enter_context(tc.tile_pool(name="zpool", bufs=2))
        opool = ctx.enter_context(tc.tile_pool(name="opool", bufs=2))
        ps = ctx.enter_context(tc.tile_pool(name="ps", bufs=2, space="PSUM"))
        cp = ctx.enter_context(tc.tile_pool(name="cp", bufs=4, space="PSUM"))

        wq_sb = wpool.tile([128, DC, HD], BF16, tag="wq")
        wk_sb = wpool.tile([128, DC, HD], BF16, tag="wk")
        wv_sb = wpool.tile([128, DC, HD], BF16, tag="wv")
        wo_sb = wpool.tile([128, HCN, D_MODEL], BF16, tag="wo")
        bq_sb = wpool.tile([128, HCN], F32, tag="bq")

        nc.sync.dma_start(wk_sb[:], wk_d[:])
        nc.sync.dma_start(wv_sb[:], wv_d[:])
        nc.sync.dma_start(wq_sb[:], wq_d[:])
        nc.sync.dma_start(wo_sb[:], wo_d[:])
        nc.sync.dma_start(bq_sb[:], bq_d[:])

        qhT = qkpool.tile([128, HCN, T], BF16, tag="qhT")  # [hd%128, hc, t]
        khT = qkpool.tile([128, HCN, T], BF16, tag="khT")
        vha = vpool.tile([128, SCN, NH, D_V + 1], BF16, tag="vha")  # [s%128, s//128, h, dv|1]
        ctxT = cpool.tile([128, HCN, T], BF16, tag="ctxT")

        nc.vector.memset(vha[:, :, :, D_V : D_V + 1], 1.0)

        exp_tiles = {}

        def kproj(hcg, tc_i):
            xt = xst.tile([128, DC, 512], BF16, tag="x", name="xk_t")
            nc.sync.dma_start(xt[:], xk_d[:, :, ts(tc_i, 512)])
            for hc in (2 * hcg, 2 * hcg + 1):
                pt = cp.tile([128, 512], F32, tag="u", name="kp_t")
                for dc in range(DC):
                    nc.tensor.matmul(
                        pt[:],
                        lhsT=wk_sb[:, dc, ts(hc, 128)],
                        rhs=xt[:, dc, :],
                        start=(dc == 0),
                        stop=(dc == DC - 1),
                    )
                nc.vector.tensor_copy(khT[:, hc, ts(tc_i, 512)], pt[:])

        def qproj(hcg, tc_i):
            xt = xst.tile([128, DC, 512], BF16, tag="x", name="xq_t")
            nc.sync.dma_start(xt[:], xq_d[:, :, ts(tc_i, 512)])
            for hc in (2 * hcg, 2 * hcg + 1):
                pt = cp.tile([128, 512], F32, tag="u", name="qp_t")
                for dc in range(DC):
                    nc.tensor.matmul(
                        pt[:],
                        lhsT=wq_sb[:, dc, ts(hc, 128)],
                        rhs=xt[:, dc, :],
                        start=(dc == 0),
                        stop=(dc == DC - 1),
                    )
                nc.vector.tensor_scalar_add(
                    qhT[:, hc, ts(tc_i, 512)], pt[:], bq_sb[:, hc : hc + 1]
                )

        def vproj(sc):
            xt = vst.tile([128, DC, 128], BF16, tag="xv", name="xv_t")
            nc.sync.dma_start(xt[:], xv_d[:, :, ts(sc, 128)])
            pv = cp.tile([128, 512], F32, tag="u", name="pv_t")
            for dc in range(DC):
                nc.tensor.matmul(
                    pv[:],
                    lhsT=xt[:, dc, :],
                    rhs=wv_sb[:, dc, :],
                    start=(dc == 0),
                    stop=(dc == DC - 1),
                )
            nc.vector.tensor_copy(
                vha[:, sc, :, 0:D_V], pv[:].rearrange("p (h d) -> p h d", d=D_V)
            )

        def scores_exp(p, qc):
            for sc in range(SCN):
                s_ps = ps.tile([128, 1024], F32, tag="ps", name="s_ps")
                for hl in range(2):
                    pb = hl * 64
                    nc.tensor.matmul(
                        s_ps[:, ts(hl, 512)],
                        lhsT=khT[pb : pb + 64, p, ts(sc, 128)],
                        rhs=qhT[pb : pb + 64, p, ts(qc, 512)],
                        start=True,
                        stop=True,
                    )
                if "peonly" in phases:
                    sv = zpool.tile([1, 8], F32, tag="sv", name="sv_t")
                    nc.vector.tensor_copy(sv[:], s_ps[0:1, 0:8])
                    exp_tiles[(p, sc, qc)] = None
                    continue
                e = expool.tile([128, 1024], BF16, tag="exp", name="exp_t")
                if "lowact" in phases and sc > 0:
                    nc.scalar.activation(
                        e[:, 0:8], s_ps[:, 0:8],
                        mybir.ActivationFunctionType.Exp, scale=0.125,
                    )
                else:
                    nc.scalar.activation(
                        e[:], s_ps[:], mybir.ActivationFunctionType.Exp, scale=0.125
                    )
                exp_tiles[(p, sc, qc)] = e

        def ctx_one(p, qc):
            for hl in range(2):
                h = 2 * p + hl
                pb = hl * 64
                ct = cp.tile([128, 512], F32, tag="u", name="ct_t")
                for sc in range(SCN):
                    e = exp_tiles[(p, sc, qc)]
                    rhs = (
                        qhT[:, p, ts(qc, 512)] if "peonly" in phases
                        else e[:, ts(hl, 512)]
                    )
                    nc.tensor.matmul(
                        ct[0 : D_V + 1, :],
                        lhsT=vha[:, sc, h, :],
                        rhs=rhs,
                        start=(sc == 0),
                        stop=(sc == SCN - 1),
                    )
                if "norecip" in phases:
                    nc.vector.tensor_copy(
                        ctxT[pb : pb + 64, p, ts(qc, 512)], ct[0:D_V, :]
                    )
                else:
                    # One immediate PSUM->SBUF copy releases the ct bank for
                    # the next unit's matmuls; the rest of the normalize chain
                    # (recip, broadcast, mul) then runs entirely from SBUF.
                    t1 = zpool.tile([D_V + 1, 512], F32, tag="t1", name="t1_t")
                    nc.vector.tensor_copy(t1[:], ct[0 : D_V + 1, :])
                    # Z to partition 0 (approx recip breaks on base!=0 / PSUM)
                    tz = zpool.tile([1, 512], F32, tag="tz", name="tz_t")
                    nc.vector.tensor_copy(tz[:], t1[D_V : D_V + 1, :])
                    rz = zpool.tile([1, 512], F32, tag="rz", name="rz_t")
                    if RECIP_MODE == "exact":
                        nc.vector.reciprocal(rz[:], tz[:])
                    else:
                        nc.vector.reciprocal_approx_fast(rz[:], tz[:])
                    bc = zpool.tile([64, 512], F32, tag="bc", name="bc_t")
                    nc.gpsimd.partition_broadcast(bc[:], rz[:], channels=64)
                    nc.vector.tensor_mul(
                        ctxT[pb : pb + 64, p, ts(qc, 512)], t1[0:D_V, :], bc[:]
                    )
            for sc in range(SCN):
                del exp_tiles[(p, sc, qc)]

        def outproj(qt):
            for d2 in range(2):
                po = cp.tile([128, 512], F32, tag="u", name="po_t")
                for hc in range(HCN):
                    nc.tensor.matmul(
                        po[:],
                        lhsT=ctxT[:, hc, ts(qt, 128)],
                        rhs=wo_sb[:, hc, ts(d2, 512)],
                        start=(hc == 0),
                        stop=(hc == HCN - 1),
                    )
                o_sb = opool.tile([128, 512], F32, tag="o", name="o_sb")
                nc.vector.tensor_copy(o_sb[:], po[:])
                nc.sync.dma_start(out_d[qt, :, ts(d2, 512)], o_sb[:])

        # ------------------- emission schedule -------------------
        kproj(0, 0)
        vproj(0)
        kproj(0, 1)
        vproj(1)
        kproj(0, 2)
        vproj(2)
        kproj(0, 3)
        vproj(3)
        qproj(0, 0)
        vproj(4)
        scores_exp(0, 0)
        kproj(1, 0)
        vproj(5)
        kproj(1, 1)
        vproj(6)
        scores_exp(1, 0)
        kproj(1, 2)
        vproj(7)
        kproj(1, 3)
        vproj(8)
        qproj(1, 0)
        for sc in range(9, SCN):
            vproj(sc)
        scores_exp(2, 0)
        ctx_one(0, 0)
        scores_exp(3, 0)
        ctx_one(1, 0)

        for qc in range(1, QCN):
            qproj(0, qc)
            scores_exp(0, qc)
            ctx_one(2, qc - 1)
            scores_exp(1, qc)
            ctx_one(3, qc - 1)
            if qc >= 2:
                for qt in range(4 * (qc - 2), 4 * (qc - 1)):
                    outproj(qt)
            qproj(1, qc)
            scores_exp(2, qc)
            ctx_one(0, qc)
            scores_exp(3, qc)
            ctx_one(1, qc)

        ctx_one(2, QCN - 1)
        ctx_one(3, QCN - 1)
        for qt in range(8, SCN):
            outproj(qt)


# ---------------------------------------------------------------------------
# host side
# ---------------------------------------------------------------------------

_NC_CACHE = {}


def _get_nc(reps: int = 1):
    if reps not in _NC_CACHE:
        _NC_CACHE[reps] = build_nc(reps)
    return _NC_CACHE[reps]


def _to_bf16(a):
    return np.ascontiguousarray(a).astype(ml_dtypes.bfloat16)


def make_in_maps(q, k, v, Wq, bq, Wk, bk, Wv, bv, Wo, bo):
    """Build the per-core input maps (host-side sharding + layout)."""
    in_maps = []
    for c in range(N_CORES):
        b = c // 2
        hg = c % 2
        hs = slice(hg * NH, hg * NH + NH)

        def xt(x):
            # (S, D) -> [p, dc, t] bf16 with D = dc*128 + p
            return _to_bf16(
                np.asarray(x, np.float32).T.reshape(DC, 128, T).transpose(1, 0, 2)
            )

        def wproj(W):
            # (8, 1024, 64) -> [p, dc, hd]  (hd = h*64+dv, D = dc*128+p)
            Wc = np.asarray(W[hs], np.float32).transpose(1, 0, 2).reshape(D_MODEL, HD)
            return _to_bf16(Wc.reshape(DC, 128, HD).transpose(1, 0, 2))

        wo_c = np.asarray(Wo[hg * HD : (hg + 1) * HD], np.float32)  # (512, 1024)
        bq_c = np.asarray(bq[hs], np.float32).reshape(HD)  # (512,)

        in_maps.append(
            {
                "xq_t": xt(q[b]),
                "xk_t": xt(k[b]),
                "xv_t": xt(v[b]),
                "wq": wproj(Wq),
                "wk": wproj(Wk),
                "wv": wproj(Wv),
                "wo": _to_bf16(wo_c.reshape(HCN, 128, D_MODEL).transpose(1, 0, 2)),
                "bq": np.ascontiguousarray(bq_c.reshape(HCN, 128).T),
            }
        )
    return in_maps


def combine_outputs(results, bv, Wo, bo):
    """results: list of 8 dicts with 'out' (16,128,1024). Returns (B,S,D)."""
    bo_eff = np.asarray(bo, np.float32) + np.asarray(bv, np.float32).reshape(-1) @ np.asarray(
        Wo, np.float32
    )
    out = np.empty((B, S, D_MODEL), np.float32)
    for b in range(B):
        p0 = results[2 * b]["out"].reshape(S, D_MODEL)
        p1 = results[2 * b + 1]["out"].reshape(S, D_MODEL)
        out[b] = p0 + p1 + bo_eff
    return out


def kernel(q, k, v, Wq, bq, Wk, bk, Wv, bv, Wo, bo):
    from concourse.bass_utils import run_bass_kernel_spmd

    nc = _get_nc(1)
    in_maps = make_in_maps(q, k, v, Wq, bq, Wk, bk, Wv, bv, Wo, bo)
    res = run_bass_kernel_spmd(nc, in_maps, core_ids=list(range(N_CORES)))
    return combine_outputs(res.results, bv, Wo, bo)
